# revision 50
# baseline (speedup 1.0000x reference)
"""DeepSeekV2-MoE Trainium2 kernel (8 NeuronCores, expert-parallel).

Strategy (v2):
  - Each core owns 2 of the 16 experts (expert-parallel sharding of
    w1_gate / w1_up / w2). The small router gate is replicated.
  - Router (logits -> top-4 -> softmax weights) is computed on every core
    in exact fp32 (top-4/5 logit gaps go down to ~6e-5, so reduced
    precision would flip expert assignments).
  - Dispatch (token compaction per expert) is done on-device with
    triangular-matmul prefix sums + one-hot compaction matmuls, all in
    fp16 (values are small integers / gate weights, fp16-exact where it
    matters) so the PE runs them at full rate with fast weight loads.
  - Token rows are gathered AND transposed in one dma_gather(transpose=
    True) from a bf16 copy of x, directly producing the [h, slot] layout
    the FFN needs (no PE transposes).
  - FFN runs in bf16 (weights + activations, fp32 PSUM accumulate),
    output is scaled by the gate weight on-device and written back as
    compact [slot, H] bf16 tiles plus the slot->token map; the host
    scatter-adds the 16 compact expert outputs into the full [T, H]
    output (combine).
"""

import sys

for _p in ("/opt/trn_rl_repo",):
    if _p not in sys.path:
        sys.path.insert(0, _p)

from contextlib import ExitStack

import numpy as np

import concourse.bacc as bacc
import concourse.bass as bass
import concourse.mybir as mybir
import concourse.tile as tile
from concourse import library_config
from concourse.bass_utils import run_bass_kernel_spmd

dt = mybir.dt

# Problem dimensions (fixed for this problem instance).
T, H, I, E, TOPK = 2048, 2048, 1024, 16, 4
NCORES, EPC = 8, 2          # 8 cores, 2 experts per core
C = 640                     # per-expert token capacity (5 * 128)
HC = H // 128               # 16 h-chunks of 128
IT = I // 128               # 8 i-tiles of 128
CWS = (384, 256)            # stage-1 token chunks (also the gather pieces)
CT = C // 128               # 5 slot tiles of 128
HN, HW_ = 4, 512            # stage-2 output h chunks (4 x 512)
ICG = I // 128              # 8 i contraction chunks

FFN_DT = dt.bfloat16        # matmul dtype for the expert FFN
DSP_DT = dt.float16         # dispatch matmul dtype (ids <= 2047 exact)


def _bc(ap, shape):
    return ap.to_broadcast(shape)


def build_program():
    """Builds the SPMD Bass/Tile program (identical on all 8 cores)."""
    nc = bacc.Bacc(
        "TRN2",
        target_bir_lowering=False,
        debug=False,
        enable_asserts=False,
        num_devices=NCORES,
    )
    f32 = dt.float32

    xt = nc.dram_tensor("xt", [H, T], f32, kind="ExternalInput").ap()
    xg = nc.dram_tensor("xg", [T, H], FFN_DT, kind="ExternalInput").ap()
    gwt = nc.dram_tensor("gwt", [128, HC * E], f32, kind="ExternalInput").ap()
    w1g = nc.dram_tensor("w1g", [EPC, IT, 128, H], FFN_DT, kind="ExternalInput").ap()
    w1u = nc.dram_tensor("w1u", [EPC, IT, 128, H], FFN_DT, kind="ExternalInput").ap()
    w2b = nc.dram_tensor("w2b", [EPC, HN, 128, ICG * HW_], FFN_DT, kind="ExternalInput").ap()
    ident = nc.dram_tensor("ident", [128, 128], f32, kind="ExternalInput").ap()
    ustrict = nc.dram_tensor("ustrict", [128, 128], DSP_DT, kind="ExternalInput").ap()
    iotac = nc.dram_tensor("iotac", [128, 2 * C], DSP_DT, kind="ExternalInput").ap()
    smalls = nc.dram_tensor("smalls", [128, 192], DSP_DT, kind="ExternalInput").ap()
    sels = nc.dram_tensor("sels", [128, 2 * E], f32, kind="ExternalInput").ap()
    smalls2 = nc.dram_tensor("smalls2", [128, 184], DSP_DT, kind="ExternalInput").ap()
    yc = nc.dram_tensor("yc", [EPC, 128, CT, HN * HW_], FFN_DT, kind="ExternalOutput").ap()
    idsout = nc.dram_tensor("idsout", [EPC, 16, C // 16], dt.int16, kind="ExternalOutput").ap()

    with tile.TileContext(nc) as tc, ExitStack() as ctx:
        consts = ctx.enter_context(tc.tile_pool(name="consts", bufs=1))
        # gwt first: the router's first matmul only needs gwt + x chunk 0.
        gwt_sb = consts.tile_from(gwt, name="gwt_sb")
        _act = mybir.EngineType.Activation
        ident_sb = consts.tile_from(ident, name="ident_sb", forced_dma_engine=_act)
        sels_sb = consts.tile_from(sels, name="sels_sb", forced_dma_engine=_act)
        # dispatch-phase consts go on the scalar HWDGE ring so they don't
        # delay the router stream on the sync ring.
        nc.gpsimd.load_library(library_config.mlp)

        # Persistent small tiles that cross phase boundaries.
        pers = ctx.enter_context(tc.tile_pool(name="pers", bufs=1))
        gates = pers.tile([128, 2 * E], f32, name="gates")  # [p, j*16+f]
        masks = pers.tile([128, 2 * E], DSP_DT, name="masks")
        ids128 = [
            pers.tile([128, C // 16], dt.int16, name=f"ids128_{j}") for j in range(EPC)
        ]
        gw2d = [pers.tile([128, CT], f32, name=f"gw2d_{j}") for j in range(EPC)]

        # ---------------- Router: logits in exact fp32 ----------------
        with tc.tile_pool(name="rxt", bufs=8) as xtp, tc.tile_pool(
            name="lps", bufs=1, space="PSUM"
        ) as lps, tc.tile_pool(name="rsb", bufs=1) as rsb:
            lpsums = [lps.tile([E, 512], f32, name=f"lps{q}") for q in range(4)]
            for hc in range(HC):
                # Two half-row tiles per block on alternating HWDGE rings:
                # separate tiles give fine-grained arrival waits (Tile dep
                # tracking is whole-tile).
                if hc == 0:
                    # Even finer first load so matmul 0 starts earliest.
                    ha = [xtp.tile([128, 512], f32, name="xq") for _ in range(2)]
                    nc.sync.dma_start(ha[0][:], xt[0:128, 0:512])
                    nc.scalar.dma_start(ha[1][:], xt[0:128, 512:1024])
                    hb = xtp.tile([128, 1024], f32, name="xh")
                    nc.sync.dma_start(hb[:], xt[0:128, 1024:T])
                    rhs_of = lambda q: (ha[q][:, 0:512] if q < 2
                                        else hb[:, (q - 2) * 512 : (q - 1) * 512])
                else:
                    h0 = xtp.tile([128, 1024], f32, name="xh")
                    h1 = xtp.tile([128, 1024], f32, name="xh")
                    (nc.sync if hc % 2 else nc.scalar).dma_start(
                        h0[:], xt[hc * 128 : (hc + 1) * 128, 0:1024]
                    )
                    (nc.scalar if hc % 2 else nc.sync).dma_start(
                        h1[:], xt[hc * 128 : (hc + 1) * 128, 1024:T]
                    )
                    halves = (h0, h1)
                    rhs_of = lambda q, _h=halves: _h[q // 2][
                        :, (q % 2) * 512 : (q % 2 + 1) * 512
                    ]
                for q in range(4):
                    nc.tensor.matmul(
                        lpsums[q][:],
                        lhsT=gwt_sb[:, hc * E : (hc + 1) * E],
                        rhs=rhs_of(q),
                        start=(hc == 0),
                        stop=(hc == HC - 1),
                    )
            ltokT = rsb.tile([E, T], f32, name="ltokT")
            for q in range(4):
                nc.vector.tensor_copy(ltokT[:, q * 512 : (q + 1) * 512], lpsums[q][:])

            # Transpose to token-major [p, f*16+e] (token t = f*128 + p).
            ltok = rsb.tile([128, 16 * E], f32, name="ltok")
            with tc.tile_pool(name="tps", bufs=2, space="PSUM") as tps:
                for f in range(16):
                    pt = tps.tile([128, E], f32, name="pt")
                    nc.tensor.transpose(
                        pt[:], ltokT[:, f * 128 : (f + 1) * 128], ident_sb[:E, :E]
                    )
                    nc.vector.tensor_copy(ltok[:, f * E : (f + 1) * E], pt[:])

            # ---------------- Top-4 + softmax over selected ----------------
            mx = rsb.tile([128, 16 * 8], f32, name="mx")
            for f in range(16):
                nc.vector.max(mx[:, f * 8 : (f + 1) * 8], ltok[:, f * E : (f + 1) * E])
            ltok3 = ltok[:].rearrange("p (f e) -> p f e", e=E)
            mx3 = mx[:].rearrange("p (f e) -> p f e", e=8)
            sh3 = [128, 16, E]

            lsh = rsb.tile([128, 16 * E], f32, name="lsh")
            nc.vector.tensor_tensor(
                lsh[:].rearrange("p (f e) -> p f e", e=E),
                ltok3,
                _bc(mx3[:, :, 0:1], sh3),
                op=mybir.AluOpType.subtract,
            )
            expp = rsb.tile([128, 16 * E], f32, name="expp")
            nc.scalar.activation(expp[:], lsh[:], mybir.ActivationFunctionType.Exp)
            selm = rsb.tile([128, 16 * E], f32, name="selm")
            nc.vector.tensor_tensor(
                selm[:].rearrange("p (f e) -> p f e", e=E),
                ltok3,
                _bc(mx3[:, :, 3:4], sh3),
                op=mybir.AluOpType.is_ge,
            )
            pm = rsb.tile([128, 16 * E], f32, name="pm")
            nc.vector.tensor_tensor(pm[:], expp[:], selm[:], op=mybir.AluOpType.mult)
            den = rsb.tile([128, 16], f32, name="den")
            nc.vector.tensor_reduce(
                den[:],
                pm[:].rearrange("p (f e) -> p f e", e=E),
                axis=mybir.AxisListType.X,
                op=mybir.AluOpType.add,
            )
            rec = rsb.tile([128, 16], f32, name="rec")
            nc.vector.reciprocal(rec[:], den[:])
            gmat = rsb.tile([128, 16 * E], f32, name="gmat")
            nc.vector.tensor_tensor(
                gmat[:].rearrange("p (f e) -> p f e", e=E),
                pm[:].rearrange("p (f e) -> p f e", e=E),
                _bc(rec[:].rearrange("p (f o) -> p f o", o=1), sh3),
                op=mybir.AluOpType.mult,
            )
            gtmp = rsb.tile([128, 16 * E], f32, name="gtmp")
            for j in range(EPC):
                nc.vector.tensor_tensor(
                    gtmp[:].rearrange("p (f e) -> p f e", e=E),
                    gmat[:].rearrange("p (f e) -> p f e", e=E),
                    _bc(
                        sels_sb[:, j * E : (j + 1) * E].rearrange(
                            "p (f e) -> p f e", f=1
                        ),
                        sh3,
                    ),
                    op=mybir.AluOpType.mult,
                )
                nc.vector.tensor_reduce(
                    gates[:, j * E : (j + 1) * E],
                    gtmp[:].rearrange("p (f e) -> p f e", e=E),
                    axis=mybir.AxisListType.X,
                    op=mybir.AluOpType.add,
                )
                nc.vector.tensor_scalar(
                    masks[:, j * E : (j + 1) * E],
                    gates[:, j * E : (j + 1) * E],
                    0.0,
                    None,
                    op0=mybir.AluOpType.is_gt,
                )

        # Dispatch-phase consts: emitted after the router stream so their
        # DMAs queue behind it on the scalar ring (needed only ~90us in).
        ustrict_sb = consts.tile_from(
            ustrict, name="ustrict_sb", forced_dma_engine=_act
        )
        iotac_sb = consts.tile_from(iotac, name="iotac_sb", forced_dma_engine=_act)
        smalls_sb = consts.tile_from(smalls, name="smalls_sb", forced_dma_engine=_act)
        smalls2_sb = consts.tile_from(
            smalls2, name="smalls2_sb", forced_dma_engine=_act
        )

        # ------------- Dispatch (both experts, fp16 matmuls) -------------
        xts_pool = ctx.enter_context(tc.tile_pool(name="xts", bufs=2))
        xts_tiles, gsems = [], []
        with tc.tile_pool(name="dsb", bufs=1) as dsb:
            # --- slot positions: exclusive prefix sum over tokens, for
            # both experts at once (cols = j*16 + f) ---
            ppx = dsb.tile([128, 2 * E], f32, name="ppx")
            with tc.tile_pool(name="chps", bufs=1, space="PSUM") as chps:
                csT_p = chps.tile([2 * E, 1], f32, name="csT_p")
                nc.tensor.matmul(
                    csT_p[:], lhsT=masks[:], rhs=smalls_sb[:, 48:49],
                    start=True, stop=True,
                )
                csT_sb = dsb.tile([2 * E, 1], DSP_DT, name="csT_sb")
                nc.vector.tensor_copy(csT_sb[:], csT_p[:])

                exr_p = chps.tile([1, 2 * E], f32, name="exr_p")
                nc.tensor.matmul(
                    exr_p[:], lhsT=csT_sb[:], rhs=smalls2_sb[:32, 152:184],
                    start=True, stop=True,
                )
                exr_sb = dsb.tile([1, 2 * E], DSP_DT, name="exr_sb")
                nc.vector.tensor_copy(exr_sb[:], exr_p[:])

                pp = chps.tile([128, 2 * E], f32, name="pp")
                nc.tensor.matmul(pp[:], lhsT=ustrict_sb[:], rhs=masks[:],
                                 start=True, stop=False)
                nc.tensor.matmul(pp[:], lhsT=smalls_sb[0:1, 64:192], rhs=exr_sb[:],
                                 start=False, stop=True)

                # ppx = pp + 4096 * (1 - mask): selected keep slot, rest >= 4096
                mneg = dsb.tile([128, 2 * E], f32, name="mneg")
                nc.vector.tensor_scalar(
                    mneg[:], masks[:], -4096.0, None, op0=mybir.AluOpType.mult
                )
                nc.vector.tensor_tensor(
                    ppx[:], mneg[:], pp[:], op=mybir.AluOpType.add
                )
                nc.vector.tensor_scalar_add(ppx[:], ppx[:], 4096.0)
            ppx16 = dsb.tile([128, 2 * E], DSP_DT, name="ppx16")
            nc.vector.tensor_copy(ppx16[:], ppx[:])
            ppx16_3 = ppx16[:].rearrange("p (j f) -> p j f", f=E)

            # --- compaction: slot -> (token id, gate weight), both experts.
            # One [2, C] accumulator per (expert, piece); lhsT is the tiny
            # [128, 2] (id, gw) pair so the one-hot streams as the wide
            # moving operand.
            tvgs = []
            for j in range(EPC):
                gj = gates[:, j * E : (j + 1) * E]
                tvg = dsb.tile([128, 32], DSP_DT, name=f"tvg{j}")
                tvg3 = tvg[:].rearrange("p (f two) -> p f two", two=2)
                nc.vector.tensor_copy(
                    tvg3[:, :, 0:1],
                    smalls_sb[:, 32:48].rearrange("p (f o) -> p f o", o=1),
                )
                nc.vector.tensor_copy(
                    tvg3[:, :, 1:2], gj.rearrange("p (f o) -> p f o", o=1)
                )
                tvgs.append(tvg)

            iotac3 = iotac_sb[:].rearrange("p (j c) -> p j c", c=C)
            with tc.tile_pool(name="dps", bufs=1, space="PSUM") as dps, \
                 tc.tile_pool(name="tps2", bufs=2, space="PSUM") as tps2, \
                 tc.tile_pool(name="efp", bufs=4) as efp:
                igs = {}
                for j in range(EPC):
                    off = 0
                    for piece, cw in enumerate(CWS):
                        igs[j, piece] = dps.tile(
                            [2, cw], f32, name=f"ig{j}_{piece}"
                        )
                        off += cw
                for f in range(16):
                    ef = efp.tile([128, 2 * C], DSP_DT, name="ef")
                    ef3 = ef[:].rearrange("p (j c) -> p j c", c=C)
                    for j, veng in ((0, nc.vector), (1, nc.vector)):
                        veng.tensor_scalar(
                            ef3[:, j, :],
                            iotac_sb[:, 0:C],
                            ppx[:, j * E + f : j * E + f + 1],
                            None,
                            op0=mybir.AluOpType.is_equal,
                        )
                    for j in range(EPC):
                        off = 0
                        for piece, cw in enumerate(CWS):
                            nc.tensor.matmul(
                                igs[j, piece][:],
                                lhsT=tvgs[j][:, 2 * f : 2 * f + 2],
                                rhs=ef3[:, j, off : off + cw],
                                start=(f == 0), stop=(f == 15),
                            )
                            off += cw

                # Per expert: transpose [2, C] back to slot-major, build the
                # wrapped int16 ids, then fire the gather immediately.
                hi8 = smalls2_sb[:, 0:8]        # [p, hi] = 1 if p//16 == hi
                sel16 = smalls2_sb[:, 8:24]     # [p, lo] = 1 if p%16 == lo
                rep = smalls2_sb[:16, 24:152]   # [k, m] = 1 if m%16 == k
                def fire_gather(j, piece, off, cw):
                    xtsp = xts_pool.tile(
                        [128, HC, cw], FFN_DT,
                        name=f"xts{j}_{piece}", tag=f"xts{piece}",
                    )
                    gsem = nc.alloc_semaphore(f"g{j}_{piece}")
                    nc.gpsimd.dma_gather(
                        out_ap=xtsp[:],
                        in_ap=xg[:],
                        idxs_ap=ids128[j][:, off // 16 : (off + cw) // 16],
                        num_idxs=cw,
                        num_idxs_reg=cw,
                        elem_size=H,
                        transpose=True,
                    ).then_inc(gsem, 16)
                    return xtsp, gsem

                for j in range(EPC):
                    igsb = dsb.tile([2, C], f32, name=f"igsb{j}")
                    for piece, cw in enumerate(CWS):
                        o = 0 if piece == 0 else CWS[0]
                        nc.vector.tensor_copy(
                            igsb[:, o : o + cw], igs[j, piece][:]
                        )
                    xts_j, gsems_j = [], []
                    for q in range(CT):
                        tq = tps2.tile([128, 2], f32, name="tq", tag="tq")
                        nc.tensor.transpose(
                            tq[:], igsb[:, q * 128 : (q + 1) * 128],
                            ident_sb[:2, :2],
                        )
                        nc.vector.tensor_copy(
                            gw2d[j][:, q : q + 1], tq[:, 1:2]
                        )
                        idsm = dsb.tile([128, 8], DSP_DT, name="idsm")
                        nc.vector.tensor_scalar(
                            idsm[:], hi8, tq[:, 0:1], None,
                            op0=mybir.AluOpType.mult,
                        )
                        wq_ps = tps2.tile([16, 8], f32, name="wq_ps", tag="wrap")
                        nc.tensor.matmul(
                            wq_ps[:], lhsT=sel16, rhs=idsm[:], start=True, stop=True
                        )
                        wq_sb = dsb.tile([16, 8], DSP_DT, name="wq_sb")
                        nc.vector.tensor_copy(wq_sb[:], wq_ps[:])
                        rep_ps = tps2.tile([128, 8], f32, name="rep_ps", tag="wrap")
                        nc.tensor.matmul(
                            rep_ps[:], lhsT=rep, rhs=wq_sb[:], start=True, stop=True
                        )
                        nc.vector.tensor_copy(
                            ids128[j][:, q * 8 : (q + 1) * 8], rep_ps[:]
                        )
                        if q == 2:
                            # ids cols 0:24 done -> gather piece A now
                            xtsp, gsem = fire_gather(j, 0, 0, CWS[0])
                            xts_j.append(xtsp)
                            gsems_j.append(gsem)
                    xtsp, gsem = fire_gather(j, 1, CWS[0], CWS[1])
                    xts_j.append(xtsp)
                    gsems_j.append(gsem)
                    # slot -> token map for the host-side combine (off the
                    # critical path, after the gathers)
                    nc.scalar.dma_start(idsout[j], ids128[j][0:16, :])
                    xts_tiles.append(xts_j)
                    gsems.append(gsems_j)

        h_pool = ctx.enter_context(tc.tile_pool(name="hall", bufs=2))
        w1_pool = ctx.enter_context(tc.tile_pool(name="w1p", bufs=3))
        w2_pool = ctx.enter_context(tc.tile_pool(name="w2p", bufs=2))
        y_pool = ctx.enter_context(tc.tile_pool(name="yp", bufs=2))
        s_pool = ctx.enter_context(tc.tile_pool(name="sp", bufs=2))

        # ---------------- FFN per expert ----------------
        for j in range(EPC):
            # --- stage 1: g/u projections + SiLU, h in SBUF (bf16) ---
            hall = h_pool.tile([128, ICG, C], FFN_DT, name="hall", tag="hall")
            with tc.tile_pool(name="s1ps", bufs=2, space="PSUM") as s1ps:
                for it in range(IT):
                    wg = w1_pool.tile([128, H], FFN_DT, name="wg", tag="wg")
                    nc.sync.dma_start(wg[:], w1g[j, it])
                    wu = w1_pool.tile([128, H], FFN_DT, name="wu", tag="wu")
                    nc.scalar.dma_start(wu[:], w1u[j, it])
                    off = 0
                    for cq, cw in enumerate(CWS):
                        xts = xts_tiles[j][cq]
                        gsem = gsems[j][cq]
                        sl = slice(off, off + cw)
                        pg = s1ps.tile([128, cw], f32, name="pg", tag=f"pg{cq}")
                        for hc in range(HC):
                            mm = nc.tensor.matmul(
                                pg[:],
                                lhsT=wg[:, hc * 128 : (hc + 1) * 128],
                                rhs=xts[:, hc, :],
                                start=(hc == 0), stop=(hc == HC - 1),
                            )
                            if hc == 0:
                                mm._wait_ge(gsem, 16)
                        pu = s1ps.tile([128, cw], f32, name="pu", tag=f"pu{cq}")
                        for hc in range(HC):
                            nc.tensor.matmul(
                                pu[:],
                                lhsT=wu[:, hc * 128 : (hc + 1) * 128],
                                rhs=xts[:, hc, :],
                                start=(hc == 0), stop=(hc == HC - 1),
                            )
                        sg = s_pool.tile([128, cw], f32, name="sg", tag=f"sg{cq}")
                        nc.scalar.activation(
                            sg[:], pg[:], mybir.ActivationFunctionType.Silu
                        )
                        nc.vector.tensor_tensor(
                            hall[:, it, sl], sg[:], pu[:], op=mybir.AluOpType.mult
                        )
                        off += cw

            # --- stage 2: down projection, gate scaling, compact output ---
            with tc.tile_pool(name="s2ps", bufs=2, space="PSUM") as s2ps:
                for hn in range(HN):
                    wb = w2_pool.tile([128, ICG * HW_], FFN_DT, name="wb", tag="w2")
                    nc.sync.dma_start(wb[:], w2b[j, hn])
                    yh = y_pool.tile([128, CT, HW_], FFN_DT, name="yh", tag="yh")
                    for ct in range(CT):
                        py = s2ps.tile([128, HW_], f32, name="py", tag="py")
                        for ic in range(ICG):
                            nc.tensor.matmul(
                                py[:],
                                lhsT=hall[:, ic, ct * 128 : (ct + 1) * 128],
                                rhs=wb[:, ic * HW_ : (ic + 1) * HW_],
                                start=(ic == 0), stop=(ic == ICG - 1),
                            )
                        nc.vector.tensor_scalar_mul(
                            yh[:, ct, :], py[:], gw2d[j][:, ct : ct + 1]
                        )
                        nc.scalar.dma_start(
                            yc[j, :, ct, hn * HW_ : (hn + 1) * HW_], yh[:, ct, :]
                        )

    nc.compile()
    return nc


def prep_inputs(x, gate_w, w1_gate, w1_up, w2):
    """Builds the 8 per-core input maps from the full problem inputs."""
    import ml_dtypes

    bf16 = ml_dtypes.bfloat16
    f16 = np.float16
    f32 = np.float32
    x2d = np.ascontiguousarray(np.asarray(x, f32).reshape(T, H))
    xt = np.ascontiguousarray(x2d.T)
    xg = np.ascontiguousarray(x2d.astype(bf16))
    gate_w = np.asarray(gate_w, f32)
    w1_gate = np.asarray(w1_gate, f32)
    w1_up = np.asarray(w1_up, f32)
    w2 = np.asarray(w2, f32)

    gwt = np.ascontiguousarray(
        gate_w.T.reshape(HC, 128, E).transpose(1, 0, 2).reshape(128, HC * E)
    )
    ident = np.eye(128, dtype=f32)
    ustrict = np.triu(np.ones((128, 128), f32), k=1).astype(f16)
    iotac = np.tile(np.arange(C, dtype=f32), (128, 2)).astype(f16)
    smalls = np.zeros((128, 192), f32)
    smalls[:16, 0:16] = np.triu(np.ones((16, 16), f32), k=1)
    smalls[:16, 16:32] = np.eye(16, dtype=f32)
    smalls[:, 32:48] = (
        np.arange(16, dtype=f32)[None, :] * 128 + np.arange(128, dtype=f32)[:, None]
    )
    smalls[:, 48] = 1.0
    smalls[:, 64:192] = 1.0
    smalls = smalls.astype(f16)
    p_idx = np.arange(128)
    smalls2 = np.zeros((128, 184), f32)
    smalls2[:, 0:8] = (p_idx[:, None] // 16 == np.arange(8)[None, :])
    smalls2[:, 8:24] = (p_idx[:, None] % 16 == np.arange(16)[None, :])
    smalls2[:16, 24:152] = (p_idx[None, :] % 16 == np.arange(16)[:, None])
    tri16 = np.triu(np.ones((16, 16), f32), k=1)
    smalls2[0:16, 152:168] = tri16
    smalls2[16:32, 168:184] = tri16
    smalls2 = smalls2.astype(f16)

    shared = dict(
        xt=xt, xg=xg, gwt=gwt, ident=ident, ustrict=ustrict,
        iotac=iotac, smalls=smalls, smalls2=smalls2,
    )

    in_maps = []
    for c in range(NCORES):
        experts = [2 * c, 2 * c + 1]
        sels = np.zeros((128, 2 * E), f32)
        w1g_b = np.empty((EPC, IT, 128, H), bf16)
        w1u_b = np.empty((EPC, IT, 128, H), bf16)
        w2_b = np.empty((EPC, HN, 128, ICG * HW_), bf16)
        for j, e in enumerate(experts):
            sels[:, j * E + e] = 1.0
            w1g_b[j] = (
                w1_gate[e].reshape(IT, 128, HC, 128).transpose(0, 3, 2, 1)
                .reshape(IT, 128, H).astype(bf16)
            )
            w1u_b[j] = (
                w1_up[e].reshape(IT, 128, HC, 128).transpose(0, 3, 2, 1)
                .reshape(IT, 128, H).astype(bf16)
            )
            w2_b[j] = (
                w2[e].reshape(HN, HW_, ICG, 128).transpose(0, 3, 2, 1)
                .reshape(HN, 128, ICG * HW_).astype(bf16)
            )
        in_maps.append(
            dict(shared, sels=sels, w1g=w1g_b, w1u=w1u_b, w2b=w2_b)
        )
    return in_maps


_NC_CACHE = []


def get_program():
    if not _NC_CACHE:
        _NC_CACHE.append(build_program())
    return _NC_CACHE[0]


# slot s = ct*128 + p; ids128 wrapped layout: slot q*128 + hi*16 + lo is
# stored at [lo, q*8 + hi] of the first 16 partitions.
_SLOT = np.arange(C)
_IDS_ROW = _SLOT % 16
_IDS_COL = (_SLOT // 128) * 8 + (_SLOT % 128) // 16


def kernel(x, gate_w, w1_gate, w1_up, w2, topk):
    assert int(topk) == TOPK
    nc = get_program()
    in_maps = prep_inputs(x, gate_w, w1_gate, w1_up, w2)
    res = run_bass_kernel_spmd(nc, in_maps, core_ids=list(range(NCORES)))
    out = np.zeros((T, H), np.float64)
    for c in range(NCORES):
        ycv = res.results[c]["yc"]        # [EPC, 128, CT, HN*HW_] bf16
        idso = res.results[c]["idsout"]   # [EPC, 16, C//16] int16
        for j in range(EPC):
            toks = idso[j][_IDS_ROW, _IDS_COL].astype(np.int64)
            ys = (
                ycv[j].astype(np.float32).transpose(1, 0, 2).reshape(C, H)
            )
            np.add.at(out, toks, ys.astype(np.float64))
    return out.astype(np.float32).reshape(1, T, H)


# revision 58
# speedup vs baseline: 1.1963x; 1.1963x over previous
"""DeepSeekV2-MoE Trainium2 kernel (8 NeuronCores, expert-parallel).

Strategy (v2):
  - Each core owns 2 of the 16 experts (expert-parallel sharding of
    w1_gate / w1_up / w2). The small router gate is replicated.
  - Router (logits -> top-4 -> softmax weights) is computed on every core
    in exact fp32 (top-4/5 logit gaps go down to ~6e-5, so reduced
    precision would flip expert assignments).
  - Dispatch (token compaction per expert) is done on-device with
    triangular-matmul prefix sums + one-hot compaction matmuls, all in
    fp16 (values are small integers / gate weights, fp16-exact where it
    matters) so the PE runs them at full rate with fast weight loads.
  - Token rows are gathered AND transposed in one dma_gather(transpose=
    True) from a bf16 copy of x, directly producing the [h, slot] layout
    the FFN needs (no PE transposes).
  - FFN runs in bf16 (weights + activations, fp32 PSUM accumulate),
    output is scaled by the gate weight on-device and written back as
    compact [slot, H] bf16 tiles plus the slot->token map; the host
    scatter-adds the 16 compact expert outputs into the full [T, H]
    output (combine).
"""

import sys

for _p in ("/opt/trn_rl_repo",):
    if _p not in sys.path:
        sys.path.insert(0, _p)

from contextlib import ExitStack

import numpy as np

import concourse.bacc as bacc
import concourse.bass as bass
import concourse.mybir as mybir
import concourse.tile as tile
from concourse import library_config
from concourse.bass_utils import run_bass_kernel_spmd

dt = mybir.dt

# Problem dimensions (fixed for this problem instance).
T, H, I, E, TOPK = 2048, 2048, 1024, 16, 4
NCORES, EPC = 8, 2          # 8 cores, 2 experts per core
C = 640                     # per-expert token capacity (5 * 128)
HC = H // 128               # 16 h-chunks of 128
IT = I // 128               # 8 i-tiles of 128
CWS = (384, 256)            # stage-1 token chunks (also the gather pieces)
CT = C // 128               # 5 slot tiles of 128
HN, HW_ = 4, 512            # stage-2 output h chunks (4 x 512)
ICG = I // 128              # 8 i contraction chunks

FFN_DT = dt.bfloat16        # matmul dtype for the expert FFN
DSP_DT = dt.float16         # dispatch matmul dtype (ids <= 2047 exact)


def _bc(ap, shape):
    return ap.to_broadcast(shape)


def build_program():
    """Builds the SPMD Bass/Tile program (identical on all 8 cores)."""
    nc = bacc.Bacc(
        "TRN2",
        target_bir_lowering=False,
        debug=False,
        enable_asserts=False,
        num_devices=NCORES,
    )
    f32 = dt.float32

    xt = nc.dram_tensor("xt", [H, T], f32, kind="ExternalInput").ap()
    xg = nc.dram_tensor("xg", [T, H], FFN_DT, kind="ExternalInput").ap()
    gwt = nc.dram_tensor("gwt", [128, HC * E], f32, kind="ExternalInput").ap()
    w1g = nc.dram_tensor("w1g", [EPC, IT, 128, H], FFN_DT, kind="ExternalInput").ap()
    w1u = nc.dram_tensor("w1u", [EPC, IT, 128, H], FFN_DT, kind="ExternalInput").ap()
    w2b = nc.dram_tensor("w2b", [EPC, HN, 128, ICG * HW_], FFN_DT, kind="ExternalInput").ap()
    ident = nc.dram_tensor("ident", [128, 128], f32, kind="ExternalInput").ap()
    ustrict = nc.dram_tensor("ustrict", [128, 128], DSP_DT, kind="ExternalInput").ap()
    iotac = nc.dram_tensor("iotac", [128, 2 * C], DSP_DT, kind="ExternalInput").ap()
    smalls = nc.dram_tensor("smalls", [128, 192], DSP_DT, kind="ExternalInput").ap()
    sels = nc.dram_tensor("sels", [128, 2 * E], f32, kind="ExternalInput").ap()
    smalls2 = nc.dram_tensor("smalls2", [128, 184], DSP_DT, kind="ExternalInput").ap()
    yc = nc.dram_tensor("yc", [EPC, 128, CT, HN * HW_], FFN_DT, kind="ExternalOutput").ap()
    idsout = nc.dram_tensor("idsout", [EPC, 16, C // 16], dt.int16, kind="ExternalOutput").ap()

    with tile.TileContext(nc) as tc, ExitStack() as ctx:
        consts = ctx.enter_context(tc.tile_pool(name="consts", bufs=1))
        # gwt first: the router's first matmul only needs gwt + x chunk 0.
        gwt_sb = consts.tile_from(gwt, name="gwt_sb")
        _act = mybir.EngineType.Activation
        ident_sb = consts.tile_from(ident, name="ident_sb", forced_dma_engine=_act)
        sels_sb = consts.tile_from(sels, name="sels_sb", forced_dma_engine=_act)
        # dispatch-phase consts go on the scalar HWDGE ring so they don't
        # delay the router stream on the sync ring.
        nc.gpsimd.load_library(library_config.mlp)

        # Persistent small tiles that cross phase boundaries.
        pers = ctx.enter_context(tc.tile_pool(name="pers", bufs=1))
        gates = pers.tile([128, 2 * E], f32, name="gates")  # [p, j*16+f]
        masks = pers.tile([128, 2 * E], DSP_DT, name="masks")
        ids128 = [
            pers.tile([128, C // 16], dt.int16, name=f"ids128_{j}") for j in range(EPC)
        ]
        gw2d = [pers.tile([128, CT], f32, name=f"gw2d_{j}") for j in range(EPC)]

        # ---------------- Router: logits in exact fp32 ----------------
        with tc.tile_pool(name="rxt", bufs=8) as xtp, tc.tile_pool(
            name="lps", bufs=1, space="PSUM"
        ) as lps, tc.tile_pool(name="rsb", bufs=1) as rsb:
            lpsums = [lps.tile([E, 512], f32, name=f"lps{q}") for q in range(4)]
            for hc in range(HC):
                # Two half-row tiles per block on alternating HWDGE rings:
                # separate tiles give fine-grained arrival waits (Tile dep
                # tracking is whole-tile).
                if hc == 0:
                    # Even finer first load so matmul 0 starts earliest.
                    ha = [xtp.tile([128, 512], f32, name="xq") for _ in range(2)]
                    nc.sync.dma_start(ha[0][:], xt[0:128, 0:512])
                    nc.scalar.dma_start(ha[1][:], xt[0:128, 512:1024])
                    hb = xtp.tile([128, 1024], f32, name="xh")
                    nc.sync.dma_start(hb[:], xt[0:128, 1024:T])
                    rhs_of = lambda q: (ha[q][:, 0:512] if q < 2
                                        else hb[:, (q - 2) * 512 : (q - 1) * 512])
                else:
                    h0 = xtp.tile([128, 1024], f32, name="xh")
                    h1 = xtp.tile([128, 1024], f32, name="xh")
                    (nc.sync if hc % 2 else nc.scalar).dma_start(
                        h0[:], xt[hc * 128 : (hc + 1) * 128, 0:1024]
                    )
                    (nc.scalar if hc % 2 else nc.sync).dma_start(
                        h1[:], xt[hc * 128 : (hc + 1) * 128, 1024:T]
                    )
                    halves = (h0, h1)
                    rhs_of = lambda q, _h=halves: _h[q // 2][
                        :, (q % 2) * 512 : (q % 2 + 1) * 512
                    ]
                for q in range(4):
                    nc.tensor.matmul(
                        lpsums[q][:],
                        lhsT=gwt_sb[:, hc * E : (hc + 1) * E],
                        rhs=rhs_of(q),
                        start=(hc == 0),
                        stop=(hc == HC - 1),
                    )
            ltokT = rsb.tile([E, T], f32, name="ltokT")
            for q in range(4):
                nc.vector.tensor_copy(ltokT[:, q * 512 : (q + 1) * 512], lpsums[q][:])

            # Transpose to token-major [p, f*16+e] (token t = f*128 + p).
            ltok = rsb.tile([128, 16 * E], f32, name="ltok")
            with tc.tile_pool(name="tps", bufs=2, space="PSUM") as tps:
                for f in range(16):
                    pt = tps.tile([128, E], f32, name="pt")
                    nc.tensor.transpose(
                        pt[:], ltokT[:, f * 128 : (f + 1) * 128], ident_sb[:E, :E]
                    )
                    nc.vector.tensor_copy(ltok[:, f * E : (f + 1) * E], pt[:])

            # ---------------- Top-4 + softmax over selected ----------------
            mx = rsb.tile([128, 16 * 8], f32, name="mx")
            for f in range(16):
                nc.vector.max(mx[:, f * 8 : (f + 1) * 8], ltok[:, f * E : (f + 1) * E])
            ltok3 = ltok[:].rearrange("p (f e) -> p f e", e=E)
            mx3 = mx[:].rearrange("p (f e) -> p f e", e=8)
            sh3 = [128, 16, E]

            lsh = rsb.tile([128, 16 * E], f32, name="lsh")
            nc.vector.tensor_tensor(
                lsh[:].rearrange("p (f e) -> p f e", e=E),
                ltok3,
                _bc(mx3[:, :, 0:1], sh3),
                op=mybir.AluOpType.subtract,
            )
            expp = rsb.tile([128, 16 * E], f32, name="expp")
            nc.scalar.activation(expp[:], lsh[:], mybir.ActivationFunctionType.Exp)
            selm = rsb.tile([128, 16 * E], f32, name="selm")
            nc.vector.tensor_tensor(
                selm[:].rearrange("p (f e) -> p f e", e=E),
                ltok3,
                _bc(mx3[:, :, 3:4], sh3),
                op=mybir.AluOpType.is_ge,
            )
            pm = rsb.tile([128, 16 * E], f32, name="pm")
            nc.vector.tensor_tensor(pm[:], expp[:], selm[:], op=mybir.AluOpType.mult)
            den = rsb.tile([128, 16], f32, name="den")
            nc.vector.tensor_reduce(
                den[:],
                pm[:].rearrange("p (f e) -> p f e", e=E),
                axis=mybir.AxisListType.X,
                op=mybir.AluOpType.add,
            )
            rec = rsb.tile([128, 16], f32, name="rec")
            nc.vector.reciprocal(rec[:], den[:])
            gmat = rsb.tile([128, 16 * E], f32, name="gmat")
            nc.vector.tensor_tensor(
                gmat[:].rearrange("p (f e) -> p f e", e=E),
                pm[:].rearrange("p (f e) -> p f e", e=E),
                _bc(rec[:].rearrange("p (f o) -> p f o", o=1), sh3),
                op=mybir.AluOpType.mult,
            )
            gtmp = rsb.tile([128, 16 * E], f32, name="gtmp")
            for j in range(EPC):
                nc.vector.tensor_tensor(
                    gtmp[:].rearrange("p (f e) -> p f e", e=E),
                    gmat[:].rearrange("p (f e) -> p f e", e=E),
                    _bc(
                        sels_sb[:, j * E : (j + 1) * E].rearrange(
                            "p (f e) -> p f e", f=1
                        ),
                        sh3,
                    ),
                    op=mybir.AluOpType.mult,
                )
                nc.vector.tensor_reduce(
                    gates[:, j * E : (j + 1) * E],
                    gtmp[:].rearrange("p (f e) -> p f e", e=E),
                    axis=mybir.AxisListType.X,
                    op=mybir.AluOpType.add,
                )
                nc.vector.tensor_scalar(
                    masks[:, j * E : (j + 1) * E],
                    gates[:, j * E : (j + 1) * E],
                    0.0,
                    None,
                    op0=mybir.AluOpType.is_gt,
                )

        # Dispatch-phase consts: emitted after the router stream so their
        # DMAs queue behind it on the scalar ring (needed only ~90us in).
        ustrict_sb = consts.tile_from(
            ustrict, name="ustrict_sb", forced_dma_engine=_act
        )
        iotac_sb = consts.tile_from(iotac, name="iotac_sb", forced_dma_engine=_act)
        smalls_sb = consts.tile_from(smalls, name="smalls_sb", forced_dma_engine=_act)
        smalls2_sb = consts.tile_from(
            smalls2, name="smalls2_sb", forced_dma_engine=_act
        )

        # ------------- Dispatch (both experts, fp16 matmuls) -------------
        xts_pool = ctx.enter_context(tc.tile_pool(name="xts", bufs=2))
        xts_tiles, gsems = [], []
        with tc.tile_pool(name="dsb", bufs=1) as dsb:
            # --- slot positions: exclusive prefix sum over tokens, for
            # both experts at once (cols = j*16 + f) ---
            ppx = dsb.tile([128, 2 * E], f32, name="ppx")
            with tc.tile_pool(name="chps", bufs=1, space="PSUM") as chps:
                csT_p = chps.tile([2 * E, 1], f32, name="csT_p")
                nc.tensor.matmul(
                    csT_p[:], lhsT=masks[:], rhs=smalls_sb[:, 48:49],
                    start=True, stop=True,
                )
                csT_sb = dsb.tile([2 * E, 1], DSP_DT, name="csT_sb")
                nc.vector.tensor_copy(csT_sb[:], csT_p[:])

                exr_p = chps.tile([1, 2 * E], f32, name="exr_p")
                nc.tensor.matmul(
                    exr_p[:], lhsT=csT_sb[:], rhs=smalls2_sb[:32, 152:184],
                    start=True, stop=True,
                )
                exr_sb = dsb.tile([1, 2 * E], DSP_DT, name="exr_sb")
                nc.vector.tensor_copy(exr_sb[:], exr_p[:])

                pp = chps.tile([128, 2 * E], f32, name="pp")
                nc.tensor.matmul(pp[:], lhsT=ustrict_sb[:], rhs=masks[:],
                                 start=True, stop=False)
                nc.tensor.matmul(pp[:], lhsT=smalls_sb[0:1, 64:192], rhs=exr_sb[:],
                                 start=False, stop=True)

                # ppx = pp + 4096 * (1 - mask): selected keep slot, rest >= 4096
                mneg = dsb.tile([128, 2 * E], f32, name="mneg")
                nc.vector.tensor_scalar(
                    mneg[:], masks[:], -4096.0, None, op0=mybir.AluOpType.mult
                )
                nc.vector.tensor_tensor(
                    ppx[:], mneg[:], pp[:], op=mybir.AluOpType.add
                )
                nc.vector.tensor_scalar_add(ppx[:], ppx[:], 4096.0)
            ppx16 = dsb.tile([128, 2 * E], DSP_DT, name="ppx16")
            nc.vector.tensor_copy(ppx16[:], ppx[:])
            ppx16_3 = ppx16[:].rearrange("p (j f) -> p j f", f=E)

            # --- compaction: slot -> (token id, gate weight), both experts.
            # One [2, C] accumulator per (expert, piece); lhsT is the tiny
            # [128, 2] (id, gw) pair so the one-hot streams as the wide
            # moving operand.
            tvgs = []
            for j in range(EPC):
                gj = gates[:, j * E : (j + 1) * E]
                tvg = dsb.tile([128, 32], DSP_DT, name=f"tvg{j}")
                tvg3 = tvg[:].rearrange("p (f two) -> p f two", two=2)
                nc.vector.tensor_copy(
                    tvg3[:, :, 0:1],
                    smalls_sb[:, 32:48].rearrange("p (f o) -> p f o", o=1),
                )
                nc.vector.tensor_copy(
                    tvg3[:, :, 1:2], gj.rearrange("p (f o) -> p f o", o=1)
                )
                tvgs.append(tvg)

            iotac3 = iotac_sb[:].rearrange("p (j c) -> p j c", c=C)
            with tc.tile_pool(name="dps", bufs=1, space="PSUM") as dps, \
                 tc.tile_pool(name="tps2", bufs=2, space="PSUM") as tps2, \
                 tc.tile_pool(name="efp", bufs=4) as efp:
                igs = {}
                for j in range(EPC):
                    off = 0
                    for piece, cw in enumerate(CWS):
                        igs[j, piece] = dps.tile(
                            [2, cw], f32, name=f"ig{j}_{piece}"
                        )
                        off += cw
                # Per expert: one-hot compaction, transpose [2, C] back to
                # slot-major, build the wrapped int16 ids, fire the gather.
                # Fully per-expert so expert 0's gather fires ~10us earlier
                # (expert 1's compares overlap expert 0's gather DMA).
                hi8 = smalls2_sb[:, 0:8]        # [p, hi] = 1 if p//16 == hi
                sel16 = smalls2_sb[:, 8:24]     # [p, lo] = 1 if p%16 == lo
                rep = smalls2_sb[:16, 24:152]   # [k, m] = 1 if m%16 == k
                def fire_gather(j, piece, off, cw):
                    xtsp = xts_pool.tile(
                        [128, HC, cw], FFN_DT,
                        name=f"xts{j}_{piece}", tag=f"xts{piece}",
                    )
                    gsem = nc.alloc_semaphore(f"g{j}_{piece}")
                    nc.gpsimd.dma_gather(
                        out_ap=xtsp[:],
                        in_ap=xg[:],
                        idxs_ap=ids128[j][:, off // 16 : (off + cw) // 16],
                        num_idxs=cw,
                        num_idxs_reg=cw,
                        elem_size=H,
                        transpose=True,
                    ).then_inc(gsem, 16)
                    return xtsp, gsem

                for j in range(EPC):
                    for f in range(16):
                        ef = efp.tile([128, C], DSP_DT, name="ef")
                        nc.vector.tensor_scalar(
                            ef[:],
                            iotac_sb[:, 0:C],
                            ppx[:, j * E + f : j * E + f + 1],
                            None,
                            op0=mybir.AluOpType.is_equal,
                        )
                        off = 0
                        for piece, cw in enumerate(CWS):
                            nc.tensor.matmul(
                                igs[j, piece][:],
                                lhsT=tvgs[j][:, 2 * f : 2 * f + 2],
                                rhs=ef[:, off : off + cw],
                                start=(f == 0), stop=(f == 15),
                            )
                            off += cw
                    igsb = dsb.tile([2, C], f32, name=f"igsb{j}")
                    for piece, cw in enumerate(CWS):
                        o = 0 if piece == 0 else CWS[0]
                        nc.vector.tensor_copy(
                            igsb[:, o : o + cw], igs[j, piece][:]
                        )
                    xts_j, gsems_j = [], []
                    for q in range(CT):
                        tq = tps2.tile([128, 2], f32, name="tq", tag="tq")
                        nc.tensor.transpose(
                            tq[:], igsb[:, q * 128 : (q + 1) * 128],
                            ident_sb[:2, :2],
                        )
                        nc.vector.tensor_copy(
                            gw2d[j][:, q : q + 1], tq[:, 1:2]
                        )
                        idsm = dsb.tile([128, 8], DSP_DT, name="idsm")
                        nc.vector.tensor_scalar(
                            idsm[:], hi8, tq[:, 0:1], None,
                            op0=mybir.AluOpType.mult,
                        )
                        wq_ps = tps2.tile([16, 8], f32, name="wq_ps", tag="wrap")
                        nc.tensor.matmul(
                            wq_ps[:], lhsT=sel16, rhs=idsm[:], start=True, stop=True
                        )
                        wq_sb = dsb.tile([16, 8], DSP_DT, name="wq_sb")
                        nc.vector.tensor_copy(wq_sb[:], wq_ps[:])
                        rep_ps = tps2.tile([128, 8], f32, name="rep_ps", tag="wrap")
                        nc.tensor.matmul(
                            rep_ps[:], lhsT=rep, rhs=wq_sb[:], start=True, stop=True
                        )
                        nc.vector.tensor_copy(
                            ids128[j][:, q * 8 : (q + 1) * 8], rep_ps[:]
                        )
                        if q == 2:
                            # ids cols 0:24 done -> gather piece A now
                            xtsp, gsem = fire_gather(j, 0, 0, CWS[0])
                            xts_j.append(xtsp)
                            gsems_j.append(gsem)
                    xtsp, gsem = fire_gather(j, 1, CWS[0], CWS[1])
                    xts_j.append(xtsp)
                    gsems_j.append(gsem)
                    # slot -> token map for the host-side combine (off the
                    # critical path, after the gathers)
                    nc.scalar.dma_start(idsout[j], ids128[j][0:16, :])
                    xts_tiles.append(xts_j)
                    gsems.append(gsems_j)

        h_pool = ctx.enter_context(tc.tile_pool(name="hall", bufs=2))
        w1_pool = ctx.enter_context(tc.tile_pool(name="w1p", bufs=3))
        w2_pool = ctx.enter_context(tc.tile_pool(name="w2p", bufs=2))
        y_pool = ctx.enter_context(tc.tile_pool(name="yp", bufs=2))
        s_pool = ctx.enter_context(tc.tile_pool(name="sp", bufs=2))

        # ---------------- FFN per expert ----------------
        # Both PSUM pools stay open across stages and experts (4 + 2*2 = 8
        # banks) so stage transitions don't wait on bank churn.
        s1ps = ctx.enter_context(tc.tile_pool(name="s1ps", bufs=1, space="PSUM"))
        s2ps = ctx.enter_context(tc.tile_pool(name="s2ps", bufs=2, space="PSUM"))
        for j in range(EPC):
            # --- stage 1: g/u projections + SiLU, h in SBUF (bf16) ---
            hall = h_pool.tile([128, ICG, C], FFN_DT, name="hall", tag="hall")
            if True:
                for it in range(IT):
                    wg = w1_pool.tile([128, H], FFN_DT, name="wg", tag="wg")
                    nc.sync.dma_start(wg[:], w1g[j, it])
                    wu = w1_pool.tile([128, H], FFN_DT, name="wu", tag="wu")
                    nc.scalar.dma_start(wu[:], w1u[j, it])
                    off = 0
                    for cq, cw in enumerate(CWS):
                        xts = xts_tiles[j][cq]
                        gsem = gsems[j][cq]
                        sl = slice(off, off + cw)
                        pg = s1ps.tile([128, cw], f32, name="pg", tag=f"pg{cq}")
                        for hc in range(HC):
                            mm = nc.tensor.matmul(
                                pg[:],
                                lhsT=wg[:, hc * 128 : (hc + 1) * 128],
                                rhs=xts[:, hc, :],
                                start=(hc == 0), stop=(hc == HC - 1),
                            )
                            if hc == 0:
                                mm._wait_ge(gsem, 16)
                        pu = s1ps.tile([128, cw], f32, name="pu", tag=f"pu{cq}")
                        for hc in range(HC):
                            nc.tensor.matmul(
                                pu[:],
                                lhsT=wu[:, hc * 128 : (hc + 1) * 128],
                                rhs=xts[:, hc, :],
                                start=(hc == 0), stop=(hc == HC - 1),
                            )
                        sg = s_pool.tile([128, cw], f32, name="sg", tag=f"sg{cq}")
                        nc.scalar.activation(
                            sg[:], pg[:], mybir.ActivationFunctionType.Silu
                        )
                        nc.vector.tensor_tensor(
                            hall[:, it, sl], sg[:], pu[:], op=mybir.AluOpType.mult
                        )
                        off += cw

            # --- stage 2: down projection, gate scaling, compact output ---
            if True:
                for hn in range(HN):
                    wb = w2_pool.tile([128, ICG * HW_], FFN_DT, name="wb", tag="w2")
                    nc.sync.dma_start(wb[:], w2b[j, hn])
                    yh = y_pool.tile([128, CT, HW_], FFN_DT, name="yh", tag="yh")
                    for ct in range(CT):
                        py = s2ps.tile([128, HW_], f32, name="py", tag="py")
                        for ic in range(ICG):
                            nc.tensor.matmul(
                                py[:],
                                lhsT=hall[:, ic, ct * 128 : (ct + 1) * 128],
                                rhs=wb[:, ic * HW_ : (ic + 1) * HW_],
                                start=(ic == 0), stop=(ic == ICG - 1),
                            )
                        nc.vector.tensor_scalar_mul(
                            yh[:, ct, :], py[:], gw2d[j][:, ct : ct + 1]
                        )
                        nc.scalar.dma_start(
                            yc[j, :, ct, hn * HW_ : (hn + 1) * HW_], yh[:, ct, :]
                        )

    nc.compile()
    return nc


def prep_inputs(x, gate_w, w1_gate, w1_up, w2):
    """Builds the 8 per-core input maps from the full problem inputs."""
    import ml_dtypes

    bf16 = ml_dtypes.bfloat16
    f16 = np.float16
    f32 = np.float32
    x2d = np.ascontiguousarray(np.asarray(x, f32).reshape(T, H))
    xt = np.ascontiguousarray(x2d.T)
    xg = np.ascontiguousarray(x2d.astype(bf16))
    gate_w = np.asarray(gate_w, f32)
    w1_gate = np.asarray(w1_gate, f32)
    w1_up = np.asarray(w1_up, f32)
    w2 = np.asarray(w2, f32)

    gwt = np.ascontiguousarray(
        gate_w.T.reshape(HC, 128, E).transpose(1, 0, 2).reshape(128, HC * E)
    )
    ident = np.eye(128, dtype=f32)
    ustrict = np.triu(np.ones((128, 128), f32), k=1).astype(f16)
    iotac = np.tile(np.arange(C, dtype=f32), (128, 2)).astype(f16)
    smalls = np.zeros((128, 192), f32)
    smalls[:16, 0:16] = np.triu(np.ones((16, 16), f32), k=1)
    smalls[:16, 16:32] = np.eye(16, dtype=f32)
    smalls[:, 32:48] = (
        np.arange(16, dtype=f32)[None, :] * 128 + np.arange(128, dtype=f32)[:, None]
    )
    smalls[:, 48] = 1.0
    smalls[:, 64:192] = 1.0
    smalls = smalls.astype(f16)
    p_idx = np.arange(128)
    smalls2 = np.zeros((128, 184), f32)
    smalls2[:, 0:8] = (p_idx[:, None] // 16 == np.arange(8)[None, :])
    smalls2[:, 8:24] = (p_idx[:, None] % 16 == np.arange(16)[None, :])
    smalls2[:16, 24:152] = (p_idx[None, :] % 16 == np.arange(16)[:, None])
    tri16 = np.triu(np.ones((16, 16), f32), k=1)
    smalls2[0:16, 152:168] = tri16
    smalls2[16:32, 168:184] = tri16
    smalls2 = smalls2.astype(f16)

    shared = dict(
        xt=xt, xg=xg, gwt=gwt, ident=ident, ustrict=ustrict,
        iotac=iotac, smalls=smalls, smalls2=smalls2,
    )

    in_maps = []
    for c in range(NCORES):
        experts = [2 * c, 2 * c + 1]
        sels = np.zeros((128, 2 * E), f32)
        w1g_b = np.empty((EPC, IT, 128, H), bf16)
        w1u_b = np.empty((EPC, IT, 128, H), bf16)
        w2_b = np.empty((EPC, HN, 128, ICG * HW_), bf16)
        for j, e in enumerate(experts):
            sels[:, j * E + e] = 1.0
            w1g_b[j] = (
                w1_gate[e].reshape(IT, 128, HC, 128).transpose(0, 3, 2, 1)
                .reshape(IT, 128, H).astype(bf16)
            )
            w1u_b[j] = (
                w1_up[e].reshape(IT, 128, HC, 128).transpose(0, 3, 2, 1)
                .reshape(IT, 128, H).astype(bf16)
            )
            w2_b[j] = (
                w2[e].reshape(HN, HW_, ICG, 128).transpose(0, 3, 2, 1)
                .reshape(HN, 128, ICG * HW_).astype(bf16)
            )
        in_maps.append(
            dict(shared, sels=sels, w1g=w1g_b, w1u=w1u_b, w2b=w2_b)
        )
    return in_maps


_NC_CACHE = []


def get_program():
    if not _NC_CACHE:
        _NC_CACHE.append(build_program())
    return _NC_CACHE[0]


# slot s = ct*128 + p; ids128 wrapped layout: slot q*128 + hi*16 + lo is
# stored at [lo, q*8 + hi] of the first 16 partitions.
_SLOT = np.arange(C)
_IDS_ROW = _SLOT % 16
_IDS_COL = (_SLOT // 128) * 8 + (_SLOT % 128) // 16


def kernel(x, gate_w, w1_gate, w1_up, w2, topk):
    assert int(topk) == TOPK
    nc = get_program()
    in_maps = prep_inputs(x, gate_w, w1_gate, w1_up, w2)
    res = run_bass_kernel_spmd(nc, in_maps, core_ids=list(range(NCORES)))
    out = np.zeros((T, H), np.float64)
    for c in range(NCORES):
        ycv = res.results[c]["yc"]        # [EPC, 128, CT, HN*HW_] bf16
        idso = res.results[c]["idsout"]   # [EPC, 16, C//16] int16
        for j in range(EPC):
            toks = idso[j][_IDS_ROW, _IDS_COL].astype(np.int64)
            ys = (
                ycv[j].astype(np.float32).transpose(1, 0, 2).reshape(C, H)
            )
            np.add.at(out, toks, ys.astype(np.float64))
    return out.astype(np.float32).reshape(1, T, H)
